# revision 28
# baseline (speedup 1.0000x reference)
"""Trainium2 Bass kernel for the lifted-structure metric loss (nn_Metric_Loss).

Even/odd decomposition: with A = T[0::2], B = T[1::2], C = S (all [2048,1024]),
the two losses need the Gram products P1=A.A', P2=A.B', P3=B.B' (loss T) and
P1, P4=A.C', P5=C.C' (loss Z) -- P1 is shared, and the symmetric products only
need their upper triangles.  In 512-blocks that is 3*10 + 2*16 = 62 block
matmuls (vs 72 for the interleaved formulation).

Per pair p (rows of A / interleaved rows 2p,2p+1):
    S_T[p] = rs1[p] + rowsumE(P2)[p] + colsumE(P2)[p] + rs3[p] - eb_T[p]
where rs* are full symmetric row sums of exp(m + P*) and the eb/dij pair
corrections are O(N d) dot products of the quantized inputs computed on the
HOST -- no on-device mask work at all.

Device (SPMD, identical program, per-core data): 4 "pairs" per core, each
pair = (lhs 512-block, 2 rhs 512-blocks) = 32 fp8 DoubleRow matmuls at the
N=512 streaming roofline (~216 ns/MM).  Per m-tile: 8 MMs into a 2-bank PSUM
tile, one [128,1024] Exp activation with row-sum accumulation, and a bf16
add tree for the column-sum tiles.  The 4th pair's last m-tile ships its two
exp tiles raw (no accumulation, no final add) so the post-matmul tail is just
two activations and small DMAs; its rhs1 is always the diagonal block, whose
column sums the host ignores.  Dummy matmuls at the head keep the PE busy
(HAM clock-gate warm-up) while the first input blocks stream in; input DMA
uses a partition-major layout so transfers run at 3 KB contiguous runs.
"""

import numpy as np
import ml_dtypes

import concourse.mybir as mybir
import concourse.tile as tile
from concourse import bacc
from concourse.bass import ds
from concourse.bass_utils import run_bass_kernel_spmd

N, D_EMB = 4096, 1024
P_ROWS = N // 2           # 2048 rows per matrix A/B/C
NCORES = 8
B = 512                   # block size
KC = D_EMB // 128         # 8 k-chunks
K2 = KC // 2              # 4 DoubleRow k-steps
MT = B // 128             # 4 m-tiles per block
NPAIR = 4                 # pairs per core
NSLOT = 3 * NPAIR         # input block slots per core
NCS = 9                   # cs slots: 6 pair cs + pair3 partial + 2 raw tiles
NRP = 15                  # accumulated row-sum columns (pairs*tiles - t3)
NDUMMY = 34               # PE warm-up matmuls (N=128) during the DMA head
MARGIN = 0.5

# The deal: per core, 4 pairs of (prod, L, r0, r1); block = (matrix, idx) with
# matrix 0=A, 1=B, 2=C.  prod: 1/3/5 = symmetric Grams of A/B/C, 2 = A.B',
# 4 = A.C', 0 = duplicated filler (host ignores).  Constraints honored:
#  - every product block covered exactly once (sym blocks in one orientation)
#  - each pair's two rhs blocks share the lhs block and host accumulator
#  - pair 3's rhs1 is always the diagonal block (col sums ignored) and its
#    rhs0 is never diagonal (its col sums are used)
PAIRS = [
 [(2,(0,0),(1,0),(1,1)), (2,(0,0),(1,2),(1,3)), (3,(1,3),(1,0),(1,1)), (3,(1,0),(1,1),(1,0))],
 [(2,(0,1),(1,0),(1,1)), (2,(0,1),(1,2),(1,3)), (2,(0,2),(1,0),(1,1)), (3,(1,1),(1,2),(1,1))],
 [(2,(0,2),(1,2),(1,3)), (2,(0,3),(1,0),(1,1)), (2,(0,3),(1,2),(1,3)), (3,(1,2),(1,0),(1,2))],
 [(1,(0,3),(0,0),(0,1)), (1,(0,0),(0,1),(0,0)), (1,(0,1),(0,2),(0,1)), (1,(0,2),(0,0),(0,2))],
 [(4,(0,0),(2,0),(2,1)), (4,(0,0),(2,2),(2,3)), (5,(2,3),(2,0),(2,1)), (5,(2,0),(2,1),(2,0))],
 [(4,(0,1),(2,0),(2,1)), (4,(0,1),(2,2),(2,3)), (4,(0,2),(2,0),(2,1)), (5,(2,1),(2,2),(2,1))],
 [(4,(0,2),(2,2),(2,3)), (4,(0,3),(2,0),(2,1)), (4,(0,3),(2,2),(2,3)), (5,(2,2),(2,0),(2,2))],
 [(1,(0,3),(0,2),(0,3)), (3,(1,3),(1,2),(1,3)), (0,(1,3),(1,0),(1,1)), (5,(2,3),(2,2),(2,3))],
]

_CACHE = {}


def _build_nc():
    nc = bacc.Bacc(
        "TRN2",
        target_bir_lowering=False,
        debug=False,
        num_devices=NCORES,
        enable_partition_id=False,
        monotonic_sem_count=0,
    )
    f32 = mybir.dt.float32
    bf16 = mybir.dt.bfloat16
    fp8 = mybir.dt.float8e4
    DR = mybir.MatmulPerfMode.DoubleRow
    EXP = mybir.ActivationFunctionType.Exp
    # partition-major input: per-partition contiguous 3 KB runs per (k2, 3
    # slots) -> DMA moves at large-element rate instead of 1 KB descriptors
    blk = nc.dram_tensor(
        "blk", [128, K2, NSLOT, 2, B], fp8, kind="ExternalInput"
    ).ap()
    out_main = nc.dram_tensor("out_main", [128, NRP], f32, kind="ExternalOutput").ap()
    out_cs = nc.dram_tensor("out_cs", [128, NCS * B], bf16, kind="ExternalOutput").ap()

    with tile.TileContext(nc) as tc:
        with (
            tc.tile_pool(name="consts", bufs=1) as consts,
            tc.tile_pool(name="psum", bufs=4, space="PSUM") as psum_pool,
            tc.tile_pool(name="esc", bufs=4) as esc_pool,
            tc.tile_pool(name="stats", bufs=3) as stats,
        ):
            bias_sb = consts.tile([128, 1], f32, tag="bias")
            nc.vector.memset(bias_sb, MARGIN)
            rp_sb = consts.tile([128, NRP], f32, tag="rp")
            # warm-up fodder for dummy matmuls; mostly garbage -- their psum
            # output is overwritten by the first start=True matmul.  Only a
            # single element is memset (the framework requires one write)
            # so the dummies start the moment the preamble ends.
            dum = consts.tile([128, 2, 128], fp8, tag="dum")
            nc.vector.memset(dum[:, 0, 0:1], 0.0)

            blk_sb = consts.tile([128, K2, NSLOT, 2, B], fp8, tag="blk")
            # pair 0 by k-step so its first matmuls gate on one 384 KB burst
            for k2 in range(K2):
                nc.sync.dma_start(
                    out=blk_sb[:, k2, 0:3], in_=blk[:, k2, 0:3]
                )
            # pair 1 in k-halves; pairs 2-3 whole
            for kh in range(2):
                nc.sync.dma_start(
                    out=blk_sb[:, 2 * kh : 2 * kh + 2, 3:6],
                    in_=blk[:, 2 * kh : 2 * kh + 2, 3:6],
                )
            for pr in range(2, NPAIR):
                nc.sync.dma_start(
                    out=blk_sb[:, :, 3 * pr : 3 * pr + 3],
                    in_=blk[:, :, 3 * pr : 3 * pr + 3],
                )

            def mm(ps_ap, pr, k2, t, r):
                nc.tensor.matmul(
                    ps_ap,
                    blk_sb[:, k2, 3 * pr, :, ds(128 * t, 128)],
                    blk_sb[:, k2, 3 * pr + 1 + r],
                    start=(k2 == 0), stop=(k2 == K2 - 1),
                    perf_mode=DR,
                )

            escs = {}
            # pairs 0-1: k-outer so each k-step's DMA burst feeds 8 matmuls
            # (their data is still streaming in when they start); the last
            # k-step runs t-major so each m-tile's activation fires before
            # the pair's matmuls end
            for pr in range(2):
                psq = [
                    psum_pool.tile([128, 2, B], f32, tag="ps", name=f"ps{pr}_{t}")
                    for t in range(MT)
                ]
                if pr == 0:
                    # dummy matmuls: keep the PE busy from the preamble on,
                    # so the HAM clock-gate is warm when real work starts
                    for _ in range(NDUMMY):
                        nc.tensor.matmul(
                            psq[0][:, 0, 0:128], dum, dum,
                            start=True, stop=True, perf_mode=DR,
                        )
                for k2 in range(K2):
                    if k2 < K2 - 1:
                        for r in range(2):
                            for t in range(MT):
                                mm(psq[t][:, r, :], pr, k2, t, r)
                    else:
                        for t in range(MT):
                            for r in range(2):
                                mm(psq[t][:, r, :], pr, k2, t, r)
                esc = esc_pool.tile([128, MT, 2, B], bf16, tag="esc")
                escs[pr] = esc
                for t in range(MT):
                    col = MT * pr + t
                    nc.scalar.activation(
                        esc[:, t], psq[t], EXP, bias=bias_sb, scale=1.0,
                        accum_out=rp_sb[:, col : col + 1],
                    )
            # pairs 2-3: t-outer, data fully resident by the time they run
            for pr in range(2, NPAIR):
                last = pr == NPAIR - 1
                esc = esc_pool.tile([128, MT, 2, B], bf16, tag="esc")
                escs[pr] = esc
                for t in range(MT):
                    if last and t == MT - 1:
                        # tail m-tile: r-outer matmuls + plain exps with no
                        # accumulation; the raw bf16 tiles ship per half and
                        # the host derives both row and col sums, so nothing
                        # slow trails the final matmuls.  Separate psum
                        # tiles so ACT(rhs0) never serializes rhs1's matmuls.
                        for r in range(2):
                            psl = psum_pool.tile(
                                [128, 2, B], f32, tag="ps", name=f"psl{r}"
                            )
                            for k2 in range(K2):
                                mm(psl[:, 0, :], pr, k2, t, r)
                            nc.scalar.activation(
                                esc[:, t, r], psl[:, 0], EXP,
                                bias=bias_sb, scale=1.0,
                            )
                    else:
                        ps = psum_pool.tile([128, 2, B], f32, tag="ps")
                        for k2 in range(K2):
                            for r in range(2):
                                mm(ps[:, r, :], pr, k2, t, r)
                        col = MT * pr + t
                        nc.scalar.activation(
                            esc[:, t], ps, EXP, bias=bias_sb, scale=1.0,
                            accum_out=rp_sb[:, col : col + 1],
                        )
            # col sums: bf16 add tree over the m-tiles (2x DVE rate); the
            # slow GpSimd add first so only DVE adds trail activations
            for pr in range(NPAIR - 1):
                esc = escs[pr]
                a01 = stats.tile([128, 2, B], bf16, tag="a01")
                nc.gpsimd.tensor_add(a01, esc[:, 0], esc[:, 1])
                a012 = stats.tile([128, 2, B], bf16, tag="a012")
                nc.vector.tensor_add(a012, a01, esc[:, 2])
                csb = stats.tile([128, 2, B], bf16, tag="csb")
                nc.vector.tensor_add(csb, a012, esc[:, 3])
                nc.sync.dma_start(
                    out=out_cs[:, 2 * B * pr : 2 * B * (pr + 1)], in_=csb
                )
            # pair 3: rhs0 partial col sums over m-tiles 0-2 (tile 3 raw)
            esc = escs[NPAIR - 1]
            a01 = stats.tile([128, B], bf16, tag="a01l")
            nc.gpsimd.tensor_add(a01, esc[:, 0, 0, :], esc[:, 1, 0, :])
            a012 = stats.tile([128, B], bf16, tag="a012l")
            nc.vector.tensor_add(a012, a01, esc[:, 2, 0, :])
            # row sums finalize before the tail: ship rp first, then the
            # partial tree, then each raw tile as its exp completes
            nc.sync.dma_start(out=out_main, in_=rp_sb)
            nc.sync.dma_start(out=out_cs[:, 6 * B : 7 * B], in_=a012)
            nc.sync.dma_start(out=out_cs[:, 7 * B : 8 * B], in_=esc[:, MT - 1, 0])
            nc.sync.dma_start(out=out_cs[:, 8 * B : 9 * B], in_=esc[:, MT - 1, 1])
    nc.compile()
    return nc


def _get_nc():
    if "nc" not in _CACHE:
        _CACHE["nc"] = _build_nc()
    return _CACHE["nc"]


def _make_in_maps(text_embeddings, shape_embeddings):
    T = np.asarray(text_embeddings, dtype=np.float32)
    S = np.asarray(shape_embeddings, dtype=np.float32)
    fp8 = ml_dtypes.float8_e4m3
    q8 = (T[0::2].astype(fp8), T[1::2].astype(fp8), S.astype(fp8))

    def xg(M8):  # [2048, 1024] -> [128, K2, 2, 2048] transposed-chunk layout
        XT = np.ascontiguousarray(M8.T)
        return XT.reshape(K2, 2, 128, P_ROWS).transpose(2, 0, 1, 3)

    G = [xg(m) for m in q8]
    in_maps = []
    for c in range(NCORES):
        blk = np.empty((128, K2, NSLOT, 2, B), dtype=fp8)
        for p, (prod, L, r0, r1) in enumerate(PAIRS[c]):
            for si, (m, i) in enumerate((L, r0, r1)):
                blk[:, :, 3 * p + si] = G[m][:, :, :, B * i : B * (i + 1)]
        in_maps.append({"blk": blk})
    return in_maps, q8


def _finalize(outs, q8):
    A, Bm, C = (m.astype(np.float64) for m in q8)
    rs = {k: np.zeros(P_ROWS, np.float64) for k in (1, 2, 3, 4, 5)}
    cs = {1: rs[1], 3: rs[3], 5: rs[5],
          2: np.zeros(P_ROWS, np.float64), 4: np.zeros(P_ROWS, np.float64)}
    for c, o in enumerate(outs):
        rp = np.asarray(o["out_main"], np.float64)           # [128, 15]
        ct = (np.asarray(o["out_cs"], np.float32).astype(np.float64)
              .reshape(128, NCS, B).transpose(1, 0, 2))      # [9, 128, 512]
        for p, (prod, L, r0, r1) in enumerate(PAIRS[c]):
            if prod == 0:
                continue
            li = L[1]
            for t in range(MT):
                g0 = B * li + 128 * t
                if p == NPAIR - 1 and t == MT - 1:
                    # tail m-tile: row sums from the raw exp tiles
                    rs[prod][g0 : g0 + 128] += (
                        ct[7].sum(axis=1) + ct[8].sum(axis=1))
                else:
                    rs[prod][g0 : g0 + 128] += rp[:, MT * p + t]
            for ri, r in enumerate((r0, r1)):
                if prod in (1, 3, 5) and r == L:
                    continue                                  # diag: rows only
                if p == NPAIR - 1:
                    if ri == 1:
                        continue                              # diag, skip
                    # partial tree (tiles 0-2) + raw tile 3
                    colsum = ct[6].sum(axis=0) + ct[7].sum(axis=0)
                else:
                    colsum = ct[2 * p + ri].sum(axis=0)       # [512]
                cs[prod][B * r[1] : B * (r[1] + 1)] += colsum
    d1 = np.einsum("ij,ij->i", A, A)
    d2 = np.einsum("ij,ij->i", A, Bm)
    d3 = np.einsum("ij,ij->i", Bm, Bm)
    d4 = np.einsum("ij,ij->i", A, C)
    d5 = np.einsum("ij,ij->i", C, C)
    m = MARGIN
    s_t = rs[1] + rs[2] + cs[2] + rs[3] - (
        np.exp(m + d1) + 2.0 * np.exp(m + d2) + np.exp(m + d3))
    s_z = rs[1] + rs[4] + cs[4] + rs[5] - (
        np.exp(m + d1) + 2.0 * np.exp(m + d4) + np.exp(m + d5))
    j_t = np.square(np.maximum(np.log(s_t) - d2, 0.0))
    j_z = np.square(np.maximum(np.log(s_z) - d4, 0.0))
    total = j_t.mean() / 2.0 + 2.0 * (j_z.mean() / 2.0)
    return np.asarray(total, dtype=np.float32)


def kernel(text_embeddings, shape_embeddings):
    in_maps, q8 = _make_in_maps(text_embeddings, shape_embeddings)
    nc = _get_nc()
    res = run_bass_kernel_spmd(nc, in_maps, core_ids=list(range(NCORES)))
    return _finalize(res.results, q8)


# revision 29
# speedup vs baseline: 1.0197x; 1.0197x over previous
"""Trainium2 Bass kernel for the lifted-structure metric loss (nn_Metric_Loss).

Even/odd decomposition: with A = T[0::2], B = T[1::2], C = S (all [2048,1024]),
the two losses need the Gram products P1=A.A', P2=A.B', P3=B.B' (loss T) and
P1, P4=A.C', P5=C.C' (loss Z) -- P1 is shared, and the symmetric products only
need their upper triangles.  In 512-blocks that is 3*10 + 2*16 = 62 block
matmuls (vs 72 for the interleaved formulation).

Per pair p (rows of A / interleaved rows 2p,2p+1):
    S_T[p] = rs1[p] + rowsumE(P2)[p] + colsumE(P2)[p] + rs3[p] - eb_T[p]
where rs* are full symmetric row sums of exp(m + P*) and the eb/dij pair
corrections are O(N d) dot products of the quantized inputs computed on the
HOST -- no on-device mask work at all.

Device (SPMD, identical program, per-core data): 4 "pairs" per core, each
pair = (lhs 512-block, 2 rhs 512-blocks) = 32 fp8 DoubleRow matmuls at the
N=512 streaming roofline (~216 ns/MM).  Per m-tile: 8 MMs into a 2-bank PSUM
tile, one [128,1024] Exp activation with row-sum accumulation, and a bf16
add tree for the column-sum tiles.  The 4th pair's last m-tile ships its two
exp tiles raw (no accumulation, no final add) so the post-matmul tail is just
two activations and small DMAs; its rhs1 is always the diagonal block, whose
column sums the host ignores.  Dummy matmuls at the head keep the PE busy
(HAM clock-gate warm-up) while the first input blocks stream in; input DMA
uses a partition-major layout so transfers run at 3 KB contiguous runs.
"""

import numpy as np
import ml_dtypes

import concourse.mybir as mybir
import concourse.tile as tile
from concourse import bacc
from concourse.bass import ds
from concourse.bass_utils import run_bass_kernel_spmd

N, D_EMB = 4096, 1024
P_ROWS = N // 2           # 2048 rows per matrix A/B/C
NCORES = 8
B = 512                   # block size
KC = D_EMB // 128         # 8 k-chunks
K2 = KC // 2              # 4 DoubleRow k-steps
MT = B // 128             # 4 m-tiles per block
NPAIR = 4                 # pairs per core
NSLOT = 3 * NPAIR         # input block slots per core
NCS = 9                   # cs slots: 6 pair cs + pair3 partial + 2 raw tiles
NRP = 15                  # accumulated row-sum columns (pairs*tiles - t3)
NDUMMY = 28               # PE warm-up matmuls (N=128) during the DMA head
MARGIN = 0.5

# The deal: per core, 4 pairs of (prod, L, r0, r1); block = (matrix, idx) with
# matrix 0=A, 1=B, 2=C.  prod: 1/3/5 = symmetric Grams of A/B/C, 2 = A.B',
# 4 = A.C', 0 = duplicated filler (host ignores).  Constraints honored:
#  - every product block covered exactly once (sym blocks in one orientation)
#  - each pair's two rhs blocks share the lhs block and host accumulator
#  - pair 3's rhs1 is always the diagonal block (col sums ignored) and its
#    rhs0 is never diagonal (its col sums are used)
PAIRS = [
 [(2,(0,0),(1,0),(1,1)), (2,(0,0),(1,2),(1,3)), (3,(1,3),(1,0),(1,1)), (3,(1,0),(1,1),(1,0))],
 [(2,(0,1),(1,0),(1,1)), (2,(0,1),(1,2),(1,3)), (2,(0,2),(1,0),(1,1)), (3,(1,1),(1,2),(1,1))],
 [(2,(0,2),(1,2),(1,3)), (2,(0,3),(1,0),(1,1)), (2,(0,3),(1,2),(1,3)), (3,(1,2),(1,0),(1,2))],
 [(1,(0,3),(0,0),(0,1)), (1,(0,0),(0,1),(0,0)), (1,(0,1),(0,2),(0,1)), (1,(0,2),(0,0),(0,2))],
 [(4,(0,0),(2,0),(2,1)), (4,(0,0),(2,2),(2,3)), (5,(2,3),(2,0),(2,1)), (5,(2,0),(2,1),(2,0))],
 [(4,(0,1),(2,0),(2,1)), (4,(0,1),(2,2),(2,3)), (4,(0,2),(2,0),(2,1)), (5,(2,1),(2,2),(2,1))],
 [(4,(0,2),(2,2),(2,3)), (4,(0,3),(2,0),(2,1)), (4,(0,3),(2,2),(2,3)), (5,(2,2),(2,0),(2,2))],
 [(1,(0,3),(0,2),(0,3)), (3,(1,3),(1,2),(1,3)), (0,(1,3),(1,0),(1,1)), (5,(2,3),(2,2),(2,3))],
]

_CACHE = {}


def _build_nc():
    nc = bacc.Bacc(
        "TRN2",
        target_bir_lowering=False,
        debug=False,
        num_devices=NCORES,
        enable_partition_id=False,
        monotonic_sem_count=0,
    )
    f32 = mybir.dt.float32
    bf16 = mybir.dt.bfloat16
    fp8 = mybir.dt.float8e4
    DR = mybir.MatmulPerfMode.DoubleRow
    EXP = mybir.ActivationFunctionType.Exp
    # partition-major input: per-partition contiguous 3 KB runs per (k2, 3
    # slots) -> DMA moves at large-element rate instead of 1 KB descriptors
    blk = nc.dram_tensor(
        "blk", [128, K2, NSLOT, 2, B], fp8, kind="ExternalInput"
    ).ap()
    out_main = nc.dram_tensor("out_main", [128, NRP], f32, kind="ExternalOutput").ap()
    out_cs = nc.dram_tensor("out_cs", [128, NCS * B], bf16, kind="ExternalOutput").ap()

    with tile.TileContext(nc) as tc:
        with (
            tc.tile_pool(name="consts", bufs=1) as consts,
            tc.tile_pool(name="psum", bufs=4, space="PSUM") as psum_pool,
            tc.tile_pool(name="esc", bufs=4) as esc_pool,
            tc.tile_pool(name="stats", bufs=3) as stats,
        ):
            bias_sb = consts.tile([128, 1], f32, tag="bias")
            nc.vector.memset(bias_sb, MARGIN)
            rp_sb = consts.tile([128, NRP], f32, tag="rp")
            # warm-up fodder for dummy matmuls; mostly garbage -- their psum
            # output is overwritten by the first start=True matmul.  Only a
            # single element is memset (the framework requires one write)
            # so the dummies start the moment the preamble ends.
            dum = consts.tile([128, 2, 128], fp8, tag="dum")
            nc.vector.memset(dum[:, 0, 0:1], 0.0)

            blk_sb = consts.tile([128, K2, NSLOT, 2, B], fp8, tag="blk")
            # pair 0 by k-step so its first matmuls gate on one 384 KB burst
            for k2 in range(K2):
                nc.sync.dma_start(
                    out=blk_sb[:, k2, 0:3], in_=blk[:, k2, 0:3]
                )
            # pair 1 in k-halves; pairs 2-3 whole
            for kh in range(2):
                nc.sync.dma_start(
                    out=blk_sb[:, 2 * kh : 2 * kh + 2, 3:6],
                    in_=blk[:, 2 * kh : 2 * kh + 2, 3:6],
                )
            for pr in range(2, NPAIR):
                nc.sync.dma_start(
                    out=blk_sb[:, :, 3 * pr : 3 * pr + 3],
                    in_=blk[:, :, 3 * pr : 3 * pr + 3],
                )

            def mm(ps_ap, pr, k2, t, r):
                nc.tensor.matmul(
                    ps_ap,
                    blk_sb[:, k2, 3 * pr, :, ds(128 * t, 128)],
                    blk_sb[:, k2, 3 * pr + 1 + r],
                    start=(k2 == 0), stop=(k2 == K2 - 1),
                    perf_mode=DR,
                )

            escs = {}
            # pairs 0-1: k-outer so each k-step's DMA burst feeds 8 matmuls
            # (their data is still streaming in when they start); the last
            # k-step runs t-major so each m-tile's activation fires before
            # the pair's matmuls end
            for pr in range(2):
                psq = [
                    psum_pool.tile([128, 2, B], f32, tag="ps", name=f"ps{pr}_{t}")
                    for t in range(MT)
                ]
                if pr == 0:
                    # dummy matmuls: keep the PE busy from the preamble on,
                    # so the HAM clock-gate is warm when real work starts
                    for _ in range(NDUMMY):
                        nc.tensor.matmul(
                            psq[0][:, 0, 0:128], dum, dum,
                            start=True, stop=True, perf_mode=DR,
                        )
                for k2 in range(K2):
                    if k2 < K2 - 1:
                        for r in range(2):
                            for t in range(MT):
                                mm(psq[t][:, r, :], pr, k2, t, r)
                    else:
                        for t in range(MT):
                            for r in range(2):
                                mm(psq[t][:, r, :], pr, k2, t, r)
                esc = esc_pool.tile([128, MT, 2, B], bf16, tag="esc")
                escs[pr] = esc
                for t in range(MT):
                    col = MT * pr + t
                    nc.scalar.activation(
                        esc[:, t], psq[t], EXP, bias=bias_sb, scale=1.0,
                        accum_out=rp_sb[:, col : col + 1],
                    )
            # pairs 2-3: t-outer, data fully resident by the time they run
            for pr in range(2, NPAIR):
                last = pr == NPAIR - 1
                esc = esc_pool.tile([128, MT, 2, B], bf16, tag="esc")
                escs[pr] = esc
                for t in range(MT):
                    if last and t == MT - 1:
                        # tail m-tile: r-outer matmuls + plain exps with no
                        # accumulation; the raw bf16 tiles ship per half and
                        # the host derives both row and col sums, so nothing
                        # slow trails the final matmuls.  Separate psum
                        # tiles so ACT(rhs0) never serializes rhs1's matmuls.
                        for r in range(2):
                            psl = psum_pool.tile(
                                [128, 2, B], f32, tag="ps", name=f"psl{r}"
                            )
                            for k2 in range(K2):
                                mm(psl[:, 0, :], pr, k2, t, r)
                            nc.scalar.activation(
                                esc[:, t, r], psl[:, 0], EXP,
                                bias=bias_sb, scale=1.0,
                            )
                    else:
                        ps = psum_pool.tile([128, 2, B], f32, tag="ps")
                        for k2 in range(K2):
                            for r in range(2):
                                mm(ps[:, r, :], pr, k2, t, r)
                        col = MT * pr + t
                        nc.scalar.activation(
                            esc[:, t], ps, EXP, bias=bias_sb, scale=1.0,
                            accum_out=rp_sb[:, col : col + 1],
                        )
            # col sums: bf16 add tree over the m-tiles (2x DVE rate); the
            # slow GpSimd add first so only DVE adds trail activations
            for pr in range(NPAIR - 1):
                esc = escs[pr]
                a01 = stats.tile([128, 2, B], bf16, tag="a01")
                nc.gpsimd.tensor_add(a01, esc[:, 0], esc[:, 1])
                a012 = stats.tile([128, 2, B], bf16, tag="a012")
                nc.vector.tensor_add(a012, a01, esc[:, 2])
                csb = stats.tile([128, 2, B], bf16, tag="csb")
                nc.vector.tensor_add(csb, a012, esc[:, 3])
                nc.sync.dma_start(
                    out=out_cs[:, 2 * B * pr : 2 * B * (pr + 1)], in_=csb
                )
            # pair 3: rhs0 partial col sums over m-tiles 0-2 (tile 3 raw)
            esc = escs[NPAIR - 1]
            a01 = stats.tile([128, B], bf16, tag="a01l")
            nc.gpsimd.tensor_add(a01, esc[:, 0, 0, :], esc[:, 1, 0, :])
            a012 = stats.tile([128, B], bf16, tag="a012l")
            nc.vector.tensor_add(a012, a01, esc[:, 2, 0, :])
            # row sums finalize before the tail: ship rp first, then the
            # partial tree, then each raw tile as its exp completes
            nc.sync.dma_start(out=out_main, in_=rp_sb)
            nc.sync.dma_start(out=out_cs[:, 6 * B : 7 * B], in_=a012)
            nc.sync.dma_start(out=out_cs[:, 7 * B : 8 * B], in_=esc[:, MT - 1, 0])
            nc.sync.dma_start(out=out_cs[:, 8 * B : 9 * B], in_=esc[:, MT - 1, 1])
    nc.compile()
    return nc


def _get_nc():
    if "nc" not in _CACHE:
        _CACHE["nc"] = _build_nc()
    return _CACHE["nc"]


def _make_in_maps(text_embeddings, shape_embeddings):
    T = np.asarray(text_embeddings, dtype=np.float32)
    S = np.asarray(shape_embeddings, dtype=np.float32)
    fp8 = ml_dtypes.float8_e4m3
    q8 = (T[0::2].astype(fp8), T[1::2].astype(fp8), S.astype(fp8))

    def xg(M8):  # [2048, 1024] -> [128, K2, 2, 2048] transposed-chunk layout
        XT = np.ascontiguousarray(M8.T)
        return XT.reshape(K2, 2, 128, P_ROWS).transpose(2, 0, 1, 3)

    G = [xg(m) for m in q8]
    in_maps = []
    for c in range(NCORES):
        blk = np.empty((128, K2, NSLOT, 2, B), dtype=fp8)
        for p, (prod, L, r0, r1) in enumerate(PAIRS[c]):
            for si, (m, i) in enumerate((L, r0, r1)):
                blk[:, :, 3 * p + si] = G[m][:, :, :, B * i : B * (i + 1)]
        in_maps.append({"blk": blk})
    return in_maps, q8


def _finalize(outs, q8):
    A, Bm, C = (m.astype(np.float64) for m in q8)
    rs = {k: np.zeros(P_ROWS, np.float64) for k in (1, 2, 3, 4, 5)}
    cs = {1: rs[1], 3: rs[3], 5: rs[5],
          2: np.zeros(P_ROWS, np.float64), 4: np.zeros(P_ROWS, np.float64)}
    for c, o in enumerate(outs):
        rp = np.asarray(o["out_main"], np.float64)           # [128, 15]
        ct = (np.asarray(o["out_cs"], np.float32).astype(np.float64)
              .reshape(128, NCS, B).transpose(1, 0, 2))      # [9, 128, 512]
        for p, (prod, L, r0, r1) in enumerate(PAIRS[c]):
            if prod == 0:
                continue
            li = L[1]
            for t in range(MT):
                g0 = B * li + 128 * t
                if p == NPAIR - 1 and t == MT - 1:
                    # tail m-tile: row sums from the raw exp tiles
                    rs[prod][g0 : g0 + 128] += (
                        ct[7].sum(axis=1) + ct[8].sum(axis=1))
                else:
                    rs[prod][g0 : g0 + 128] += rp[:, MT * p + t]
            for ri, r in enumerate((r0, r1)):
                if prod in (1, 3, 5) and r == L:
                    continue                                  # diag: rows only
                if p == NPAIR - 1:
                    if ri == 1:
                        continue                              # diag, skip
                    # partial tree (tiles 0-2) + raw tile 3
                    colsum = ct[6].sum(axis=0) + ct[7].sum(axis=0)
                else:
                    colsum = ct[2 * p + ri].sum(axis=0)       # [512]
                cs[prod][B * r[1] : B * (r[1] + 1)] += colsum
    d1 = np.einsum("ij,ij->i", A, A)
    d2 = np.einsum("ij,ij->i", A, Bm)
    d3 = np.einsum("ij,ij->i", Bm, Bm)
    d4 = np.einsum("ij,ij->i", A, C)
    d5 = np.einsum("ij,ij->i", C, C)
    m = MARGIN
    s_t = rs[1] + rs[2] + cs[2] + rs[3] - (
        np.exp(m + d1) + 2.0 * np.exp(m + d2) + np.exp(m + d3))
    s_z = rs[1] + rs[4] + cs[4] + rs[5] - (
        np.exp(m + d1) + 2.0 * np.exp(m + d4) + np.exp(m + d5))
    j_t = np.square(np.maximum(np.log(s_t) - d2, 0.0))
    j_z = np.square(np.maximum(np.log(s_z) - d4, 0.0))
    total = j_t.mean() / 2.0 + 2.0 * (j_z.mean() / 2.0)
    return np.asarray(total, dtype=np.float32)


def kernel(text_embeddings, shape_embeddings):
    in_maps, q8 = _make_in_maps(text_embeddings, shape_embeddings)
    nc = _get_nc()
    res = run_bass_kernel_spmd(nc, in_maps, core_ids=list(range(NCORES)))
    return _finalize(res.results, q8)
